# revision 14
# baseline (speedup 1.0000x reference)
"""MoE (8 experts, top-2) Trainium2 kernel — expert-parallel across 8 NeuronCores.

Strategy (v4, bf16, class-balanced slots):
- Host: replicate the reference router bit-exactly (jax CPU), build per-expert
  token lists, gather tokens, pre-swizzle weights to bf16.
- Each core runs TWO half-passes with different programs (uniform across
  cores, so still one SPMD program):
    half0 ("G1", 5 token tiles): a slice of a class-0/2 expert.
      up -> M1 FULL [F,F] -> sigma1 (LN/gelu blend + B-fold) -> down.
      No M2/sigma2 (classes 0/2 don't have a second spec matmul).
    half1 ("G2", 4 token tiles): a slice of a class-1/3 expert.
      up -> M1 HALF [F,F2] (class1 stage A; zero weights for class3)
      -> gelu + CT -> M2 [F2,F] -> sigma2 (LN blend fold) -> down.
  Compared to the naive uniform program (full M1 + M2 on all 9 tiles)
  this removes ~25% of the matmul cycles with zero cross-core traffic.
- Slot capacities: class-0/2 experts get 2 cores x 5 tiles = 1280 tokens;
  class-1/3 experts get 2 cores x 4 tiles = 1024. Tokens beyond capacity
  (3 tokens at seed 0) are computed exactly on the host and added during
  the unshard — correctness holds for any routing distribution.
- All matmuls bf16; biases are all zero for this problem (asserted).
  All intermediates stay in SBUF; 6 rotating PSUM accumulator banks + 2
  transpose-quad banks so accumulation groups never stall on drains.
- Host: scatter-add per-expert rows + residual (the "unshard").
"""

import os
import numpy as np

B, S, H, F, E, K = 2, 2048, 1024, 4096, 8, 2
F2 = F // 2
T = B * S
P = 128
NT = 9                         # 5 G1 tiles + 4 G2 tiles per core
CAP = NT * P                   # 1152 rows of xg/y per core
T1, T2 = 5, 4                  # tiles per half
KH = H // P                    # 8
MF = F // P                    # 32
K2 = F2 // P                   # 16
NF = F // 512                  # 8 (512-wide output chunks)
NF2 = F2 // 512                # 4
ND = H // 512                  # 2
EPS = 1e-5

# G1 slots host class-0/2 experts (2 cores x 5 tiles = 1280 tokens each),
# G2 slots host class-1/3 experts (2 cores x 4 tiles = 1024 tokens each).
G1_EXPERT = [0, 0, 2, 2, 4, 4, 6, 6]
G1_TILE0 = [0, 5, 0, 5, 0, 5, 0, 5]
G2_EXPERT = [1, 1, 5, 5, 3, 3, 7, 7]
G2_TILE0 = [0, 4, 0, 4, 0, 4, 0, 4]
CLASS_OF_EXPERT = [0, 1, 2, 3, 0, 1, 2, 3]

_CACHED_NC = None


def _build_nc():
    import concourse.mybir as mybir
    import concourse.tile as tile
    from concourse import bacc
    from concourse.masks import make_identity

    f32, bf16, AF = mybir.dt.float32, mybir.dt.bfloat16, mybir.ActivationFunctionType
    ALU = mybir.AluOpType
    nc = bacc.Bacc(num_devices=8)

    xg_e = nc.declare_dram_parameter("xg", [CAP, H], f32, isOutput=False)
    wv_e = nc.declare_dram_parameter("wv", [P, NT], f32, isOutput=False)
    flg_e = nc.declare_dram_parameter("flg", [P, 16], f32, isOutput=False)
    upw_e = nc.declare_dram_parameter("upw", [2, MF, P, KH, P], bf16, isOutput=False)
    upb_e = nc.declare_dram_parameter("upb", [P, 2, MF], f32, isOutput=False)
    w1_e = nc.declare_dram_parameter("w1", [NF, MF, P, 512], bf16, isOutput=False)
    w1h_e = nc.declare_dram_parameter("w1h", [NF2, MF, P, 512], bf16, isOutput=False)
    w2_e = nc.declare_dram_parameter("w2", [NF, K2, P, 512], bf16, isOutput=False)
    dw_e = nc.declare_dram_parameter("dw", [2, ND, MF, P, 512], bf16, isOutput=False)
    y_e = nc.declare_dram_parameter("y", [CAP, H], f32, isOutput=True)

    with tile.TileContext(nc) as tc:
        with tc.tile_pool(name="cst", bufs=1) as cst, \
             tc.tile_pool(name="sb", bufs=2) as sb, \
             tc.tile_pool(name="stat", bufs=6) as stp, \
             tc.tile_pool(name="slab", bufs=6) as slp, \
             tc.tile_pool(name="bigA", bufs=2) as bigA, \
             tc.tile_pool(name="bigB", bufs=2) as bigB, \
             tc.tile_pool(name="bigD", bufs=5) as bigD, \
             tc.tile_pool(name="ps", bufs=1, space="PSUM") as psp:

            def acc_tile(name):
                # 6 rotating accumulator banks: a fresh accumulation group
                # starts in a spare bank instead of waiting on the previous
                # group's PSUM drain
                return psp.tile([P, 512], f32, space="PSUM", tag="acc",
                                bufs=6, name=name)

            def tq_tile(name):
                # 2 rotating banks for transpose quads (4x [P,P] per bank)
                return psp.tile([P, 4, P], bf16, space="PSUM", tag="tq",
                                bufs=2, name=name)

            ident = cst.tile([P, P], bf16)
            make_identity(nc, ident[:])
            wv = cst.tile([P, NT], f32)
            nc.sync.dma_start(out=wv[:], in_=wv_e.ap())
            flg = cst.tile([P, 16], f32)
            nc.sync.dma_start(out=flg[:], in_=flg_e.ap())
            upb = cst.tile([P, 2, MF], f32)
            nc.sync.dma_start(out=upb[:], in_=upb_e.ap())
            eps_t = cst.tile([P, 1], f32)
            nc.vector.memset(eps_t[:], EPS)
            # flag columns per half h (h*8+i): 0 alpha1, 1 alpha2, 2 tauB,
            # 3 tauR, 4 tauH
            FL = lambda h, i: flg[:, h * 8 + i:h * 8 + i + 1]
            # identity pre-scaled by half0's tauB for the B-fold transposes
            identB = cst.tile([P, P], bf16)
            nc.vector.tensor_scalar(out=identB[:], in0=ident[:],
                                    scalar1=FL(0, 2), scalar2=None, op0=ALU.mult)

            def ln_coeffs(src_stats, alpha, negshift_out, scale_out):
                """src_stats [P,2] (mean,var) -> scale=a*rstd+(1-a),
                shift=-a*mean*rstd (per-partition)."""
                rstd = stp.tile([P, 1], f32, tag="rstd")
                nc.scalar.activation(out=rstd[:], in_=src_stats[:, 1:2],
                                     func=AF.Sqrt, bias=eps_t[:, 0:1])
                nc.vector.reciprocal(out=rstd[:], in_=rstd[:])
                nc.vector.tensor_scalar(out=scale_out[:], in0=rstd[:],
                                        scalar1=alpha, scalar2=None, op0=ALU.mult)
                one_m = stp.tile([P, 1], f32, tag="onem")
                nc.vector.tensor_scalar(out=one_m[:], in0=alpha, scalar1=-1.0,
                                        scalar2=1.0, op0=ALU.mult, op1=ALU.add)
                nc.vector.tensor_tensor(out=scale_out[:], in0=scale_out[:],
                                        in1=one_m[:], op=ALU.add)
                nc.vector.tensor_tensor(out=negshift_out[:], in0=src_stats[:, 0:1],
                                        in1=rstd[:], op=ALU.mult)
                nc.vector.tensor_scalar(out=negshift_out[:], in0=negshift_out[:],
                                        scalar1=-1.0, scalar2=None, op0=ALU.mult)
                nc.vector.tensor_tensor(out=negshift_out[:], in0=negshift_out[:],
                                        in1=alpha, op=ALU.mult)

            def input_ln(half, t0, tc_, nxT):
                """Load x tiles, layernorm, transpose into nxT (bf16)."""
                for t in range(tc_):
                    gt = t0 + t
                    xt = sb.tile([P, H], f32, tag="xt")
                    nc.sync.dma_start(out=xt[:], in_=xg_e.ap()[gt * P:(gt + 1) * P, :])
                    st1 = stp.tile([P, 2, nc.vector.BN_STATS_DIM], f32, tag="st1")
                    for s in range(2):
                        nc.vector.bn_stats(out=st1[:, s, :],
                                           in_=xt[:, s * 512:(s + 1) * 512])
                    mv = stp.tile([P, nc.vector.BN_AGGR_DIM], f32, tag="mv")
                    nc.vector.bn_aggr(out=mv[:], in_=st1[:])
                    rstd = stp.tile([P, 1], f32, tag="rstd0")
                    nc.scalar.activation(out=rstd[:], in_=mv[:, 1:2], func=AF.Sqrt,
                                         bias=eps_t[:, 0:1])
                    nc.vector.reciprocal(out=rstd[:], in_=rstd[:])
                    nshift = stp.tile([P, 1], f32, tag="nsh0")
                    nc.vector.tensor_tensor(out=nshift[:], in0=mv[:, 0:1], in1=rstd[:],
                                            op=ALU.mult)
                    nc.vector.tensor_scalar(out=nshift[:], in0=nshift[:], scalar1=-1.0,
                                            scalar2=None, op0=ALU.mult)
                    nxt = sb.tile([P, H], bf16, tag="nxt")
                    nc.scalar.activation(out=nxt[:], in_=xt[:], func=AF.Identity,
                                         bias=nshift[:, 0:1], scale=rstd[:, 0:1])
                    for q in range(KH // 4):
                        tq = tq_tile(f"tqx_{half}_{t}_{q}")
                        for j in range(4):
                            k = q * 4 + j
                            nc.tensor.transpose(out=tq[:, j, :],
                                                in_=nxt[:, k * P:(k + 1) * P],
                                                identity=ident[:])
                        nc.vector.tensor_copy(
                            out=nxT[:, q * 4:(q + 1) * 4, t * P:(t + 1) * P],
                            in_=tq[:])

            def up_proj(half, tc_, nxT, h1T, phases):
                # phases: list of group-lists. The first phase covers only
                # tile 0's columns, so the PE starts on it while later tiles
                # are still in layernorm (group-outer avoids head-of-line
                # blocking in the in-order PE queue). Costs one extra uslab
                # fetch pass per additional phase.
                for pi, groups in enumerate(phases):
                    for m in range(MF):
                        uslab = slp.tile([P, KH, P], bf16, tag="uslab")
                        nc.sync.dma_start(out=uslab[:], in_=upw_e.ap()[half, m])
                        for g0, gw in groups:
                            up_ps = acc_tile(f"upps_{half}_{pi}_{m}_{g0}")
                            for k in range(KH):
                                nc.tensor.matmul(out=up_ps[:, 0:gw],
                                                 lhsT=uslab[:, k, :],
                                                 rhs=nxT[:, k, g0:g0 + gw],
                                                 start=(k == 0), stop=(k == KH - 1))
                            nc.scalar.activation(out=h1T[:, m, g0:g0 + gw],
                                                 in_=up_ps[:, 0:gw], func=AF.Gelu,
                                                 bias=upb[:, half, m:m + 1])

            def matmul_block(name, nf, kk, lhsT_src, w_ap, tc_, evac):
                """Accumulating matmul: out chunk n over kk k-tiles, per tile."""
                for n in range(nf):
                    ps_list = [acc_tile(f"{name}_{n}_{i}") for i in range(tc_)]
                    for k in range(kk):
                        wslab = slp.tile([P, 512], bf16, tag="wslab")
                        nc.sync.dma_start(out=wslab[:], in_=w_ap[n, k])
                        for t in range(tc_):
                            nc.tensor.matmul(out=ps_list[t][:],
                                             lhsT=lhsT_src(k, t),
                                             rhs=wslab[:], start=(k == 0),
                                             stop=(k == kk - 1))
                    for t in range(tc_):
                        evac(n, t, ps_list[t])

            def down_proj(half, t0, tc_, h1T):
                for n in range(ND):
                    ps_list = [acc_tile(f"dacc_{half}_{n}_{i}") for i in range(tc_)]
                    for k in range(MF):
                        dslab = slp.tile([P, 512], bf16, tag="wslab")
                        nc.sync.dma_start(out=dslab[:], in_=dw_e.ap()[half, n, k])
                        for t in range(tc_):
                            nc.tensor.matmul(out=ps_list[t][:],
                                             lhsT=h1T[:, k, t * P:(t + 1) * P],
                                             rhs=dslab[:], start=(k == 0),
                                             stop=(k == MF - 1))
                    for t in range(tc_):
                        gt = t0 + t
                        yv = sb.tile([P, 512], f32, tag="yv")
                        nc.scalar.activation(out=yv[:], in_=ps_list[t][:],
                                             func=AF.Copy, scale=wv[:, gt:gt + 1])
                        nc.sync.dma_start(out=y_e.ap()[gt * P:(gt + 1) * P,
                                                       n * 512:(n + 1) * 512],
                                          in_=yv[:])

            # ================= half0: G1 (class 0/2 expert, 5 tiles) ======
            nxT = bigB.tile([P, K2, T1 * P], bf16, tag="nxT", name="nxT_0")
            input_ln(0, 0, T1, nxT)
            h1T = bigA.tile([P, MF, T1 * P], bf16, tag="h1T", name="h1T_0")
            # phase 1 needs only tile 0's layernorm; phase 2 the rest
            up_proj(0, T1, nxT, h1T,
                    [[(0, 128)], [(128, 256), (384, 256)]])

            A_tiles = [bigD.tile([P, F], bf16, tag="A", name=f"A_0_{t}")
                       for t in range(T1)]
            stA = [stp.tile([P, NF, nc.vector.BN_STATS_DIM], f32, tag=f"stA_{t}",
                            name=f"stA_0_{t}") for t in range(T1)]

            def evac_m1(n, t, ps):
                nc.vector.bn_stats(out=stA[t][:, n, :], in_=ps[:])
                nc.vector.tensor_copy(out=A_tiles[t][:, n * 512:(n + 1) * 512],
                                      in_=ps[:])
            matmul_block("m1", NF, MF,
                         lambda k, t: h1T[:, k, t * P:(t + 1) * P],
                         w1_e.ap(), T1, evac_m1)

            # sigma1 (G1): G = gelu(blend-LN(A)); h1T = tauH*h1T + tauB*G.T
            for t in range(T1):
                mvA = stp.tile([P, nc.vector.BN_AGGR_DIM], f32, tag="mvA")
                nc.vector.bn_aggr(out=mvA[:], in_=stA[t][:])
                sc1 = stp.tile([P, 1], f32, tag="sc1")
                sh1 = stp.tile([P, 1], f32, tag="sh1")
                ln_coeffs(mvA, FL(0, 0), sh1, sc1)
                At = A_tiles[t]
                for c in range(2):
                    nc.scalar.activation(out=At[:, c * 2048:(c + 1) * 2048],
                                         in_=At[:, c * 2048:(c + 1) * 2048],
                                         func=AF.Gelu, bias=sh1[:, 0:1],
                                         scale=sc1[:, 0:1])
                ts_ = slice(t * P, (t + 1) * P)
                nc.vector.tensor_scalar(out=h1T[:, :, ts_], in0=h1T[:, :, ts_],
                                        scalar1=FL(0, 4), scalar2=None, op0=ALU.mult)
                for q in range(MF // 4):
                    tq = tq_tile(f"tqb_0_{t}_{q}")
                    for j in range(4):
                        k = q * 4 + j
                        nc.tensor.transpose(out=tq[:, j, :],
                                            in_=At[:, k * P:(k + 1) * P],
                                            identity=identB[:])
                    ks = slice(q * 4, (q + 1) * 4)
                    nc.vector.tensor_tensor(out=h1T[:, ks, ts_],
                                            in0=h1T[:, ks, ts_], in1=tq[:],
                                            op=ALU.add)

            down_proj(0, 0, T1, h1T)

            # ================= half1: G2 (class 1/3 expert, 4 tiles) ======
            nxT = bigB.tile([P, K2, T1 * P], bf16, tag="nxT", name="nxT_1")
            input_ln(1, T1, T2, nxT)
            h1T = bigA.tile([P, MF, T1 * P], bf16, tag="h1T", name="h1T_1")
            # phase 1 needs only tile 0's layernorm; phase 2 the rest
            up_proj(1, T2, nxT, h1T,
                    [[(0, 128)], [(128, 128), (256, 256)]])

            A_tiles = [bigD.tile([P, F], bf16, tag="A", name=f"A_1_{t}")
                       for t in range(T2)]
            stA = [stp.tile([P, NF, nc.vector.BN_STATS_DIM], f32, tag=f"stA_{t}",
                            name=f"stA_1_{t}") for t in range(T2)]

            def evac_m1h(n, t, ps):
                nc.vector.bn_stats(out=stA[t][:, n, :], in_=ps[:])
                nc.vector.tensor_copy(out=A_tiles[t][:, n * 512:(n + 1) * 512],
                                      in_=ps[:])
            matmul_block("m1h", NF2, MF,
                         lambda k, t: h1T[:, k, t * P:(t + 1) * P],
                         w1h_e.ap(), T2, evac_m1h)

            # sigma1-lite (G2): G = gelu(blend-LN(A[:, :F2])); CT = G.T;
            # h1T *= tauH (tauB == 0 for classes 1/3 — no B-fold)
            CT = bigB.tile([P, K2, T1 * P], bf16, tag="nxT", name="CT_1")
            for t in range(T2):
                mvA = stp.tile([P, nc.vector.BN_AGGR_DIM], f32, tag="mvA")
                nc.vector.bn_aggr(out=mvA[:], in_=stA[t][:, 0:NF2, :])
                sc1 = stp.tile([P, 1], f32, tag="sc1")
                sh1 = stp.tile([P, 1], f32, tag="sh1")
                ln_coeffs(mvA, FL(1, 0), sh1, sc1)
                At = A_tiles[t]
                nc.scalar.activation(out=At[:, 0:F2], in_=At[:, 0:F2],
                                     func=AF.Gelu, bias=sh1[:, 0:1],
                                     scale=sc1[:, 0:1])
                ts_ = slice(t * P, (t + 1) * P)
                nc.vector.tensor_scalar(out=h1T[:, :, ts_], in0=h1T[:, :, ts_],
                                        scalar1=FL(1, 4), scalar2=None, op0=ALU.mult)
                for q in range(K2 // 4):
                    tq = tq_tile(f"tqc_1_{t}_{q}")
                    for j in range(4):
                        k = q * 4 + j
                        nc.tensor.transpose(out=tq[:, j, :],
                                            in_=At[:, k * P:(k + 1) * P],
                                            identity=ident[:])
                    nc.vector.tensor_copy(
                        out=CT[:, q * 4:(q + 1) * 4, ts_], in_=tq[:])

            # M2 (G2): R = G @ W2 -> A tiles (full width) + stats
            stR = [stp.tile([P, NF, nc.vector.BN_STATS_DIM], f32, tag=f"stA_{t}",
                            name=f"stR_1_{t}") for t in range(T2)]

            def evac_m2(n, t, ps):
                nc.vector.bn_stats(out=stR[t][:, n, :], in_=ps[:])
                nc.vector.tensor_copy(out=A_tiles[t][:, n * 512:(n + 1) * 512],
                                      in_=ps[:])
            matmul_block("m2", NF, K2,
                         lambda k, t: CT[:, k, t * P:(t + 1) * P],
                         w2_e.ap(), T2, evac_m2)

            # sigma2 (G2): h1T += tauR * blend-LN(R).T
            for t in range(T2):
                mv2 = stp.tile([P, nc.vector.BN_AGGR_DIM], f32, tag="mv2")
                nc.vector.bn_aggr(out=mv2[:], in_=stR[t][:])
                sc2 = stp.tile([P, 1], f32, tag="sc2")
                sh2 = stp.tile([P, 1], f32, tag="sh2")
                ln_coeffs(mv2, FL(1, 1), sh2, sc2)
                nc.vector.tensor_tensor(out=sc2[:], in0=sc2[:], in1=FL(1, 3),
                                        op=ALU.mult)
                nc.vector.tensor_tensor(out=sh2[:], in0=sh2[:], in1=FL(1, 3),
                                        op=ALU.mult)
                Rt = A_tiles[t]
                for c in range(2):
                    nc.vector.tensor_scalar(out=Rt[:, c * 2048:(c + 1) * 2048],
                                            in0=Rt[:, c * 2048:(c + 1) * 2048],
                                            scalar1=sc2[:, 0:1], scalar2=sh2[:, 0:1],
                                            op0=ALU.mult, op1=ALU.add)
                ts_ = slice(t * P, (t + 1) * P)
                for q in range(MF // 4):
                    tq = tq_tile(f"tqr_1_{t}_{q}")
                    for j in range(4):
                        k = q * 4 + j
                        nc.tensor.transpose(out=tq[:, j, :],
                                            in_=Rt[:, k * P:(k + 1) * P],
                                            identity=ident[:])
                    ks = slice(q * 4, (q + 1) * 4)
                    nc.vector.tensor_tensor(out=h1T[:, ks, ts_],
                                            in0=h1T[:, ks, ts_], in1=tq[:],
                                            op=ALU.add)

            down_proj(1, T1, T2, h1T)
    nc.finalize()
    return nc


def _routing(x_flat, ln_g, ln_b, router_w):
    """Bit-exact replication of the reference router on jax CPU."""
    import jax
    import jax.numpy as jnp
    cpu = jax.devices("cpu")[0]
    with jax.default_device(cpu):
        x = jnp.asarray(np.asarray(x_flat))
        g = jnp.asarray(np.asarray(ln_g))
        b = jnp.asarray(np.asarray(ln_b))
        rw = jnp.asarray(np.asarray(router_w))
        m = jnp.mean(x, axis=-1, keepdims=True)
        v = jnp.var(x, axis=-1, keepdims=True)
        nx = (x - m) / jnp.sqrt(v + 1e-5) * g + b
        logits = nx @ rw
        probs = jax.nn.softmax(logits, axis=-1)
        _, idx = jax.lax.top_k(probs, K)
        mask = jnp.sum(jax.nn.one_hot(idx, probs.shape[-1], dtype=probs.dtype), axis=1)
        w = probs * mask
        w = w / jnp.sum(w, axis=-1, keepdims=True)
        return np.asarray(w), np.asarray(mask), np.asarray(nx)


def _col128(vec, n):
    """[n*128] -> [128, n] partition-major layout."""
    return np.ascontiguousarray(vec.reshape(n, P).T)


def _expert_host(nx_rows, e, params):
    """Exact f32 single-expert forward for capacity-overflow tokens."""
    import jax
    import jax.numpy as jnp
    from jax.scipy.special import erf
    (up_W, down_W, spec0_W, spec1a_W, spec1b_W, spec2_W) = params
    cpu = jax.devices("cpu")[0]
    gelu = lambda v: v * 0.5 * (1 + erf(v / np.sqrt(2).astype(np.float32)))
    ln = lambda v: (v - v.mean(-1, keepdims=True)) / jnp.sqrt(
        v.var(-1, keepdims=True) + 1e-5)
    cls = CLASS_OF_EXPERT[e]
    j = 0 if e < 4 else 1
    with jax.default_device(cpu):
        h1 = gelu(jnp.asarray(nx_rows) @ up_W[e])
        if cls == 0:
            h2 = gelu(ln(h1 @ spec0_W[j]))
        elif cls == 1:
            h2 = ln(gelu(h1 @ spec1a_W[j]) @ spec1b_W[j])
        elif cls == 2:
            h2 = gelu(h1 @ spec2_W[j])
        else:
            h2 = h1
        return np.asarray(h2 @ down_W[e])


def kernel(**inputs):
    import ml_dtypes
    from concourse.bass_utils import run_bass_kernel_spmd

    bfloat16 = ml_dtypes.bfloat16

    global _CACHED_NC
    x = np.asarray(inputs["hidden_states"], np.float32)
    x_flat = x.reshape(T, H)
    w_all, mask, nx_host = _routing(x_flat, inputs["ln_g"], inputs["ln_b"],
                                    inputs["router_w"])

    up_W = np.asarray(inputs["up_W"], np.float32)
    up_b = np.asarray(inputs["up_b"], np.float32)
    down_W = np.asarray(inputs["down_W"], np.float32)
    spec0_W = np.asarray(inputs["spec0_W"], np.float32)
    spec1a_W = np.asarray(inputs["spec1a_W"], np.float32)
    spec1b_W = np.asarray(inputs["spec1b_W"], np.float32)
    spec2_W = np.asarray(inputs["spec2_W"], np.float32)
    # all spec/down biases and ln affines are trivial for this problem — fold:
    assert np.all(np.asarray(inputs["ln0_g"]) == 1) and np.all(np.asarray(inputs["ln0_b"]) == 0)
    assert np.all(np.asarray(inputs["ln1_g"]) == 1) and np.all(np.asarray(inputs["ln1_b"]) == 0)
    assert np.all(np.asarray(inputs["spec0_b"]) == 0)
    assert np.all(np.asarray(inputs["spec1a_b"]) == 0)
    assert np.all(np.asarray(inputs["spec1b_b"]) == 0)
    assert np.all(np.asarray(inputs["spec2_b"]) == 0)
    assert np.all(np.asarray(inputs["down_b"]) == 0)

    def swz(wmat, nf, kk):
        # [kk*128, nf*512] -> [nf, kk, 128, 512] bf16
        r = wmat.reshape(kk, P, nf, 512)
        return np.ascontiguousarray(r.transpose(2, 0, 1, 3)).astype(bfloat16)

    def upswz(e):
        return np.ascontiguousarray(
            up_W[e].reshape(KH, P, MF, P).transpose(2, 1, 0, 3)).astype(bfloat16)

    # per-expert token lists + device capacity split
    tok_of = []
    host_extra = []   # (expert, token-indices) beyond device capacity
    for e in range(8):
        tok = np.nonzero(mask[:, e] > 0)[0]
        cap_e = 1280 if CLASS_OF_EXPERT[e] in (0, 2) else 1024
        tok_of.append(tok[:cap_e])
        if len(tok) > cap_e:
            host_extra.append((e, tok[cap_e:]))

    def flags_of(e):
        cls = CLASS_OF_EXPERT[e]
        # al1, al2, tauB, tauR, tauH
        return {0: (1.0, 0.0, 1.0, 0.0, 0.0),
                1: (0.0, 1.0, 0.0, 1.0, 0.0),
                2: (0.0, 0.0, 1.0, 0.0, 0.0),
                3: (0.0, 0.0, 0.0, 0.0, 1.0)}[cls]

    in_maps = []
    core_rows = []    # (token-index array, valid-count) per core, CAP rows
    for c in range(8):
        e1, o1 = G1_EXPERT[c], G1_TILE0[c]
        e2, o2 = G2_EXPERT[c], G2_TILE0[c]
        cls2 = CLASS_OF_EXPERT[e2]
        j1 = 0 if e1 < 4 else 1
        j2 = 0 if e2 < 4 else 1

        idx = np.zeros(CAP, np.int64)
        wvv = np.zeros(CAP, np.float32)
        rows_valid = np.zeros(CAP, bool)
        exp_of_row = np.empty(CAP, np.int64)
        exp_of_row[:T1 * P] = e1
        exp_of_row[T1 * P:] = e2
        for (ee, oo, r0, ntl) in ((e1, o1, 0, T1), (e2, o2, T1 * P, T2)):
            tk = tok_of[ee][oo * P:oo * P + ntl * P]
            idx[r0:r0 + len(tk)] = tk
            wvv[r0:r0 + len(tk)] = w_all[tk, ee]
            rows_valid[r0:r0 + len(tk)] = True
        core_rows.append((idx, rows_valid))
        xg = x_flat[idx] * rows_valid[:, None]

        W1 = np.asarray(spec0_W[j1] if CLASS_OF_EXPERT[e1] == 0 else spec2_W[j1],
                        np.float32)
        W1h = np.zeros((F, F2), np.float32)
        W2 = np.zeros((F2, F), np.float32)
        if cls2 == 1:
            W1h[:] = spec1a_W[j2]
            W2[:] = spec1b_W[j2]

        flg = np.zeros((P, 16), np.float32)
        flg[:, 0:5] = np.array(flags_of(e1), np.float32)
        flg[:, 8:13] = np.array(flags_of(e2), np.float32)

        in_maps.append({
            "xg": xg.astype(np.float32),
            "wv": _col128(wvv, NT),
            "flg": flg,
            "upw": np.stack([upswz(e1), upswz(e2)]),
            "upb": np.stack([_col128(up_b[e1], MF), _col128(up_b[e2], MF)], 1),
            "w1": swz(W1, NF, MF),
            "w1h": swz(W1h, NF2, MF),
            "w2": swz(W2, NF, K2),
            "dw": np.stack([swz(down_W[e1], ND, MF), swz(down_W[e2], ND, MF)]),
        })

    if _CACHED_NC is None:
        _CACHED_NC = _build_nc()
    trace = os.environ.get("BASS_MOE_TRACE") == "1"
    res = run_bass_kernel_spmd(_CACHED_NC, in_maps, list(range(8)), trace=trace)
    global LAST_RES
    LAST_RES = res

    y = x_flat.copy()
    for c in range(8):
        idx, valid = core_rows[c]
        yv = res.results[c]["y"]
        # the same token may appear in both halves of one core (its two
        # routed experts); accumulate per half so fancy-index += never sees
        # duplicate indices (which numpy would collapse to a single add)
        for r0, r1 in ((0, T1 * P), (T1 * P, CAP)):
            v = valid[r0:r1]
            y[idx[r0:r1][v]] += yv[r0:r1][v]
    # capacity-overflow tokens: exact host compute
    params = (up_W, down_W, spec0_W, spec1a_W, spec1b_W, spec2_W)
    for e, tks in host_extra:
        out = _expert_host(nx_host[tks], e, params)
        y[tks] += out * w_all[tks, e:e + 1]
    return y.reshape(B, S, H)
